# revision 1
# baseline (speedup 1.0000x reference)
"""Trainium2 Bass kernel for nn_CustomMoEBranch (moe_routing).

Contract: kernel(**inputs) takes the FULL unsharded inputs (as produced by
setup_inputs) and returns the FULL [64, 192, 1024] float32 output.

Strategy: data-parallel over batch across 8 NeuronCores (8 samples each).
Each core computes the STFT->MLP gating on-device (windowed DFT as matmuls),
selects top-2 experts per sample (vector max/max_index), gathers only those
two experts' conv weights via indirect DMA, and runs the two selected
expert branches (conv k=3/5/7 stride 2 -> relu -> conv k=3 stride 2 -> relu)
as TensorE matmuls. The softmax gate weight is folded into the first conv's
ReLU epilogue (w>=0 so w*relu(z) = relu(w*z)), and the two experts' outputs
are summed on-chip with an [I;I] matmul.
"""
import sys
if '/opt/trn_rl_repo' not in sys.path:
    sys.path.insert(0, '/opt/trn_rl_repo')
import numpy as np

import concourse.bass as bass
import concourse.mybir as mybir
import concourse.tile as tile
from concourse import bacc
from concourse.bass_utils import run_bass_kernel_spmd

FP32 = mybir.dt.float32
U32 = mybir.dt.uint32
AF = mybir.ActivationFunctionType
ALU = mybir.AluOpType

N_FFT = 256
HOP = 64
E = 8
L = 4096
L1 = 2048   # conv1 out length
L2 = 1024   # conv2 out length
NF = 65     # stft frames
NCOL = 4104  # padded xcol length
KS = (3, 5, 7)

# W_all_A layout: per expert a [64, 603] block:
#  cols 0..575   : conv2 lhsT blocks, (br,d) at col (br*3+d)*64, [c_in, c_out]
#  cols 576..578 : conv2 bias bb per branch (col 576+br, row c)
#  cols 579..602 : conv1 blocks transposed: col 579+br*8+t, row c:
#                  t<7 -> conv1 weight for im2col row t; t==7 -> conv1 bias ba
CA = 603


def host_prep_consts(inputs):
    """Host-side constant tensors shared by all cores."""
    n = np.arange(N_FFT)
    win = (0.5 - 0.5 * np.cos(2.0 * np.pi * n / N_FFT)).astype(np.float64)
    q = np.arange(129)
    ang = 2.0 * np.pi * np.outer(n, q) / N_FFT  # [256, 129]
    dc = (win[:, None] * np.cos(ang)).astype(np.float32)  # [256, 129]
    ds = (win[:, None] * np.sin(ang)).astype(np.float32)
    consts = {
        "DCa": np.ascontiguousarray(dc[:128, :128]),
        "DCb": np.ascontiguousarray(dc[128:, :128]),
        "DSa": np.ascontiguousarray(ds[:128, :128]),
        "DSb": np.ascontiguousarray(ds[128:, :128]),
        "DNa": np.ascontiguousarray(dc[:128, 128:129]),
        "DNb": np.ascontiguousarray(dc[128:, 128:129]),
    }
    Wg1s = (inputs["Wg1"] / NF).astype(np.float32)  # fold 1/65 mean into Wg1
    consts["Wg1a"] = np.ascontiguousarray(Wg1s[:128])          # [128, 256]
    consts["Wg1b"] = np.ascontiguousarray(Wg1s[128:129])       # [1, 256]
    consts["bg1t"] = np.ascontiguousarray(
        np.stack([inputs["bg1"][:128], inputs["bg1"][128:]], axis=1))  # [128,2]
    consts["Wg2a"] = np.ascontiguousarray(inputs["Wg2"][:128])   # [128,128]
    consts["Wg2b"] = np.ascontiguousarray(inputs["Wg2"][128:])   # [128,128]
    consts["bg2c"] = np.ascontiguousarray(inputs["bg2"][:, None])  # [128,1]
    consts["Wg3"] = np.ascontiguousarray(inputs["Wg3"])          # [128,8]
    consts["bg3r"] = np.ascontiguousarray(inputs["bg3"][None, :])  # [1,8]
    ist = np.concatenate([np.eye(64), np.eye(64)], axis=0).astype(np.float32)
    consts["IST"] = ist                                          # [128,64]
    consts["I64"] = np.eye(64, dtype=np.float32)                 # [64,64]

    # W_all_A  [E*64, 603]
    wa = np.zeros((E, 64, CA), dtype=np.float32)
    for br, k in enumerate(KS):
        wb = inputs["wb%d" % k]   # [E, 64, 64, 3]
        for d in range(3):
            # lhsT block [c_in, c_out] = wb[e, c_out, c_in, d]
            wa[:, :, (br * 3 + d) * 64:(br * 3 + d + 1) * 64] = \
                np.transpose(wb[:, :, :, d], (0, 2, 1))
        wa[:, :, 576 + br] = inputs["bb%d" % k]  # [E, 64]
        w1 = inputs["wa%d" % k]   # [E, 64, 1, k]
        off = 3 - k // 2
        for dd in range(k):
            wa[:, :, 579 + br * 8 + off + dd] = w1[:, :, 0, dd]
        wa[:, :, 579 + br * 8 + 7] = inputs["ba%d" % k]  # conv1 bias row
    consts["WAF"] = np.ascontiguousarray(wa.reshape(E * 64, CA))
    return consts


def host_prep_core(x_core):
    """Per-core input tensors. x_core: [S, 4096]."""
    S = x_core.shape[0]
    x_ext = np.zeros((S, NCOL), dtype=np.float32)
    x_ext[:, 3:3 + L] = x_core
    xcol = np.zeros((S, 8, NCOL), dtype=np.float32)
    for d in range(7):
        xcol[:, d, :NCOL - d] = x_ext[:, d:]
    xcol[:, 7, :] = 1.0
    xr = np.pad(x_core, ((0, 0), (128, 128)), mode="reflect")
    f_idx = np.arange(NF) * HOP
    n_idx = np.arange(128)
    fr = np.zeros((S, 2, 128, NF), dtype=np.float32)
    for h in range(2):
        fr[:, h] = xr[:, (f_idx[None, :] + 128 * h + n_idx[:, None])]
    return {"xcol": xcol, "fr": fr}


def build(SPC=8, REPS=1):
    """Build the bass module. SPC = samples per core."""
    nc = bacc.Bacc("TRN2", target_bir_lowering=False, debug=False)

    d_in = {}
    for name, shape in [
        ("DCa", (128, 128)), ("DCb", (128, 128)), ("DSa", (128, 128)),
        ("DSb", (128, 128)), ("DNa", (128, 1)), ("DNb", (128, 1)),
        ("Wg1a", (128, 256)), ("Wg1b", (1, 256)), ("bg1t", (128, 2)),
        ("Wg2a", (128, 128)), ("Wg2b", (128, 128)), ("bg2c", (128, 1)),
        ("Wg3", (128, 8)), ("bg3r", (1, 8)), ("IST", (128, 64)),
        ("I64", (64, 64)), ("WAF", (E * 64, CA)),
        ("xcol", (SPC, 8, NCOL)), ("fr", (SPC, 2, 128, NF)),
    ]:
        d_in[name] = nc.dram_tensor(name, list(shape), FP32, kind="ExternalInput")
    out_d = nc.dram_tensor("out", [SPC, 192, L2], FP32, kind="ExternalOutput")

    with tile.TileContext(nc) as tc:
        with tc.tile_pool(name="consts", bufs=1) as cpool:
            ct = {}
            for name in ["DCa", "DCb", "DSa", "DSb", "DNa", "DNb", "Wg1a",
                         "Wg1b", "bg1t", "Wg2a", "Wg2b", "bg2c", "Wg3",
                         "bg3r", "IST", "I64"]:
                t = cpool.tile(list(d_in[name].shape), FP32, tag=name)
                nc.sync.dma_start(t[:], d_in[name][:])
                ct[name] = t
            ones18 = cpool.tile([1, 8], FP32, tag="ones18")
            nc.vector.memset(ones18[:], 1.0)
            iota64 = cpool.tile([128, 1], U32, tag="iota64")
            nc.gpsimd.iota(iota64[:], pattern=[[0, 1]], base=0,
                           channel_multiplier=1)
            nc.vector.tensor_scalar(iota64[:], iota64[:], 63, None,
                                    ALU.bitwise_and)

            # tiles that live for the whole kernel
            with tc.tile_pool(name="gout", bufs=1) as gpool:
                for rep in range(REPS):
                    build_rep(nc, tc, d_in, out_d, ct, gpool, iota64, SPC,
                              rep)
    nc.compile()
    return nc


def build_rep(nc, tc, d_in, out_d, ct, gpool, iota64, SPC, rep):
    r = f"r{rep}"
    # ---------------- gating ----------------
    pooled = gpool.tile([128, SPC], FP32, tag="pooled" + r)
    pooledN = gpool.tile([1, SPC], FP32, tag="pooledN" + r)
    W_Bs = gpool.tile([128, SPC], FP32, tag="W_Bs" + r)
    OFFu = gpool.tile([128, SPC], U32, tag="OFFu" + r)

    with tc.tile_pool(name="gwork", bufs=2) as gw, \
         tc.tile_pool(name="gpsum", bufs=2, space="PSUM") as gp, \
         tc.tile_pool(name="gpsum1", bufs=1, space="PSUM") as gp1:
        for s in range(SPC):
            FR = gw.tile([128, 2 * NF], FP32, tag="FR")
            nc.sync.dma_start(FR[:, 0:NF], d_in["fr"][s, 0])
            nc.sync.dma_start(FR[:, NF:2 * NF], d_in["fr"][s, 1])
            psumG = gp.tile([128, 3 * NF], FP32, tag="psumG")
            nc.tensor.matmul(psumG[:, 0:NF], ct["DCa"][:], FR[:, 0:NF],
                             start=True, stop=False)
            nc.tensor.matmul(psumG[:, 0:NF], ct["DCb"][:], FR[:, NF:2 * NF],
                             start=False, stop=True)
            nc.tensor.matmul(psumG[:, NF:2 * NF], ct["DSa"][:], FR[:, 0:NF],
                             start=True, stop=False)
            nc.tensor.matmul(psumG[:, NF:2 * NF], ct["DSb"][:], FR[:, NF:2 * NF],
                             start=False, stop=True)
            psumN = psumG[0:1, 2 * NF:3 * NF]
            nc.tensor.matmul(psumN, ct["DNa"][0:128, 0:1], FR[:, 0:NF],
                             start=True, stop=False)
            nc.tensor.matmul(psumN, ct["DNb"][0:128, 0:1], FR[:, NF:2 * NF],
                             start=False, stop=True)
            CS = gw.tile([128, 2 * NF], FP32, tag="CS")
            nc.scalar.copy(CS[:], psumG[:, 0:2 * NF])
            m2 = gw.tile([128, NF], FP32, tag="m2")
            nc.vector.tensor_tensor(out=m2[:], in0=CS[:, 0:NF],
                                    in1=CS[:, 0:NF], op=ALU.mult)
            s2 = gw.tile([128, NF], FP32, tag="s2")
            nc.vector.tensor_tensor(out=s2[:], in0=CS[:, NF:2 * NF],
                                    in1=CS[:, NF:2 * NF], op=ALU.mult)
            nc.vector.tensor_tensor(out=m2[:], in0=m2[:], in1=s2[:], op=ALU.add)
            mag = gw.tile([128, NF], FP32, tag="mag")
            nc.scalar.activation(mag[:], m2[:], AF.Sqrt)
            nc.vector.tensor_reduce(pooled[:, s:s + 1], mag[:],
                                    axis=mybir.AxisListType.X, op=ALU.add)
            magN = gw.tile([1, NF], FP32, tag="magN")
            nc.scalar.activation(magN[:], psumN, AF.Abs)
            nc.vector.tensor_reduce(pooledN[:, s:s + 1], magN[:],
                                    axis=mybir.AxisListType.X, op=ALU.add)

        # MLP
        h1p = gp1.tile([128, 2 * SPC], FP32, tag="h1p")
        for mh in range(2):
            sl = slice(mh * SPC, (mh + 1) * SPC)
            nc.tensor.matmul(h1p[:, sl], ct["Wg1a"][:, mh * 128:(mh + 1) * 128],
                             pooled[:], start=True, stop=False)
            nc.tensor.matmul(h1p[:, sl], ct["Wg1b"][:, mh * 128:(mh + 1) * 128],
                             pooledN[:], start=False, stop=True)
        h1 = gw.tile([128, 2 * SPC], FP32, tag="h1")
        for mh in range(2):
            sl = slice(mh * SPC, (mh + 1) * SPC)
            nc.scalar.activation(h1[:, sl], h1p[:, sl], AF.Relu,
                                 bias=ct["bg1t"][:, mh:mh + 1])
        h2p = gp1.tile([128, SPC], FP32, tag="h2p")
        nc.tensor.matmul(h2p[:], ct["Wg2a"][:], h1[:, 0:SPC],
                         start=True, stop=False)
        nc.tensor.matmul(h2p[:], ct["Wg2b"][:], h1[:, SPC:2 * SPC],
                         start=False, stop=True)
        h2 = gw.tile([128, SPC], FP32, tag="h2")
        nc.scalar.activation(h2[:], h2p[:], AF.Relu, bias=ct["bg2c"][:, 0:1])
        lgp = gp1.tile([SPC, 8], FP32, tag="lgp")
        nc.tensor.matmul(lgp[:], h2[:], ct["Wg3"][:], start=True, stop=False)
        nc.tensor.matmul(lgp[:], ones_ap(nc, tc, gw, SPC), ct["bg3r"][:],
                         start=False, stop=True)
        LT = gw.tile([SPC, 8], FP32, tag="LT")
        nc.vector.tensor_copy(LT[:], lgp[:])

        # top-2
        vals8 = gw.tile([SPC, 8], FP32, tag="vals8")
        inds8 = gw.tile([SPC, 8], U32, tag="inds8")
        nc.vector.max(vals8[:], LT[:])
        nc.vector.max_index(inds8[:], vals8[:], LT[:])
        idxf = gw.tile([SPC, 2], FP32, tag="idxf")
        nc.vector.tensor_copy(idxf[:], inds8[:, 0:2])
        # softmax over top-2: w0 = 1/(1+e), w1 = e/(1+e), e = exp(v1-v0)
        dv = gw.tile([SPC, 1], FP32, tag="dv")
        nc.vector.tensor_tensor(out=dv[:], in0=vals8[:, 1:2],
                                in1=vals8[:, 0:1], op=ALU.subtract)
        ev = gw.tile([SPC, 1], FP32, tag="ev")
        nc.scalar.activation(ev[:], dv[:], AF.Exp)
        ev1 = gw.tile([SPC, 1], FP32, tag="ev1")
        nc.vector.tensor_scalar_add(ev1[:], ev[:], 1.0)
        wv = gw.tile([SPC, 2], FP32, tag="wv")
        nc.vector.reciprocal(wv[:, 0:1], ev1[:])
        nc.vector.tensor_tensor(out=wv[:, 1:2], in0=ev[:], in1=wv[:, 0:1],
                                op=ALU.mult)

        # broadcast w/idx across partitions: [128, SPC]
        psumB = gp1.tile([128, SPC], FP32, tag="psumB")
        E8 = ct["I64"][0:SPC, 0:SPC]
        for j in range(2):
            nc.tensor.matmul(psumB[64 * j:64 * (j + 1), :],
                             wv[:, j:j + 1].to_broadcast([SPC, 64]), E8,
                             start=True, stop=True)
        nc.vector.tensor_copy(W_Bs[:], psumB[:])
        psumI = gp1.tile([128, SPC], FP32, tag="psumI")
        for j in range(2):
            nc.tensor.matmul(psumI[64 * j:64 * (j + 1), :],
                             idxf[:, j:j + 1].to_broadcast([SPC, 64]), E8,
                             start=True, stop=True)
        IDXB = gw.tile([128, SPC], FP32, tag="IDXB")
        nc.vector.tensor_copy(IDXB[:], psumI[:])
        nc.vector.tensor_copy(OFFu[:], IDXB[:])  # fp32 -> u32
        nc.vector.tensor_scalar(OFFu[:], OFFu[:], 6, None,
                                ALU.logical_shift_left)
        nc.vector.tensor_tensor(out=OFFu[:], in0=OFFu[:],
                                in1=iota64[:].to_broadcast([128, SPC]),
                                op=ALU.add)

    # ---------------- expert main loop ----------------
    with tc.tile_pool(name="xc", bufs=2) as xcp, \
         tc.tile_pool(name="wa", bufs=2) as wap, \
         tc.tile_pool(name="w1", bufs=2) as w1p, \
         tc.tile_pool(name="hh", bufs=6) as hhp, \
         tc.tile_pool(name="rr", bufs=4) as rrp, \
         tc.tile_pool(name="oo", bufs=3) as oop, \
         tc.tile_pool(name="bb", bufs=2) as bbp, \
         tc.tile_pool(name="ps1", bufs=2, space="PSUM") as ps1, \
         tc.tile_pool(name="ps2", bufs=2, space="PSUM") as ps2, \
         tc.tile_pool(name="psO", bufs=1, space="PSUM") as psO, \
         tc.tile_pool(name="psT", bufs=1, space="PSUM") as psT:
        for s in range(SPC):
            # gather this sample's two expert weight blocks
            wA = wap.tile([128, CA], FP32, tag="wA")
            nc.gpsimd.indirect_dma_start(
                out=wA[:], out_offset=None, in_=d_in["WAF"][:],
                in_offset=bass.IndirectOffsetOnAxis(ap=OFFu[:, s:s + 1], axis=0))
            # conv2 bias * gate weight
            bbw = bbp.tile([128, 3], FP32, tag="bbw")
            nc.vector.tensor_tensor(out=bbw[:], in0=wA[:, 576:579],
                                    in1=W_Bs[:, s:s + 1].to_broadcast([128, 3]),
                                    op=ALU.mult)
            # conv1 weights: transpose [64, 8] blocks -> [8, 64]
            W1 = w1p.tile([8, 384], FP32, tag="W1")
            for br in range(3):
                for j in range(2):
                    pT = psT.tile([8, 64], FP32, tag="pT")
                    nc.tensor.transpose(
                        pT[:], wA[64 * j:64 * (j + 1),
                                  579 + br * 8:579 + (br + 1) * 8],
                        ct["IST"][64 * j:64 * (j + 1), :])
                    nc.vector.tensor_copy(W1[:, br * 128 + 64 * j:
                                             br * 128 + 64 * (j + 1)], pT[:])
            # xcol load
            XC = xcp.tile([8, NCOL], FP32, tag="XC")
            nc.sync.dma_start(XC[:], d_in["xcol"][s])

            for br in range(3):
                # conv1: h' = relu(w * (conv1(x)+ba))  [128, 2048] + pad cols
                H = hhp.tile([128, 2 + L1], FP32, tag="H")
                nc.vector.memset(H[:, 0:1], 0.0)
                nc.vector.memset(H[:, 1 + L1:2 + L1], 0.0)
                for c in range(4):
                    p1 = ps1.tile([128, 512], FP32, tag="p1")
                    nc.tensor.matmul(
                        p1[:], W1[:, br * 128:(br + 1) * 128],
                        XC[:, 1024 * c:1024 * c + 1024:2],
                        start=True, stop=True)
                    dst = H[:, 1 + 512 * c:1 + 512 * (c + 1)]
                    if c % 2 == 0:
                        nc.scalar.activation(dst, p1[:], AF.Relu,
                                             scale=W_Bs[:, s:s + 1])
                    else:
                        nc.vector.tensor_scalar(dst, p1[:],
                                                W_Bs[:, s:s + 1], 0.0,
                                                ALU.mult, ALU.max)
                # conv2 + relu + slot-sum
                O = oop.tile([128, 512], FP32, tag="O")
                pO = psO.tile([128, 512], FP32, tag="pO")
                for c in range(2):
                    p2a = ps2.tile([128, 512], FP32, tag="p2a")
                    p2b = ps2.tile([128, 512], FP32, tag="p2b")
                    for d in range(3):
                        for j, p2 in ((0, p2a), (1, p2b)):
                            sl = slice(64 * j, 64 * (j + 1))
                            nc.tensor.matmul(
                                p2[sl, :],
                                wA[sl, (br * 3 + d) * 64:(br * 3 + d + 1) * 64],
                                H[sl, d + 1024 * c:d + 1024 * c + 1024:2],
                                start=(d == 0), stop=(d == 2),
                                tile_position=(64 * j, 64 * j))
                    R = rrp.tile([128, 512], FP32, tag="R")
                    nc.scalar.activation(R[0:64, :], p2a[0:64, :], AF.Relu,
                                         bias=bbw[0:64, br:br + 1])
                    nc.vector.tensor_scalar(R[64:128, :], p2b[64:128, :],
                                            bbw[64:128, br:br + 1], 0.0,
                                            ALU.add, ALU.max)
                    nc.tensor.matmul(pO[64 * c:64 * (c + 1), :], ct["IST"][:],
                                     R[:], start=True, stop=True,
                                     tile_position=(0, 64 * c))
                if br % 2 == 0:
                    nc.vector.tensor_copy(O[:], pO[:])
                else:
                    nc.scalar.copy(O[:], pO[:])
                for cc in range(2):
                    nc.sync.dma_start(
                        out_d[s, br * 64:(br + 1) * 64,
                              512 * cc:512 * (cc + 1)],
                        O[64 * cc:64 * (cc + 1), :])


def ones_ap(nc, tc, gw, SPC):
    t = gw.tile([1, SPC], FP32, tag="ones1S")
    nc.vector.memset(t[:], 1.0)
    return t[:]


N_CORES = 8
_cache = {}


def _get_module(SPC, REPS=1):
    key = (SPC, REPS)
    if key not in _cache:
        _cache[key] = build(SPC=SPC, REPS=REPS)
    return _cache[key]


def make_in_maps(inputs):
    consts = host_prep_consts(inputs)
    in_maps = []
    for c in range(N_CORES):
        m = dict(consts)
        m.update(host_prep_core(inputs["x"][8 * c:8 * (c + 1)]))
        in_maps.append(m)
    return in_maps


def kernel(**inputs):
    inputs = {k: np.ascontiguousarray(np.asarray(v, dtype=np.float32))
              for k, v in inputs.items()}
    nc = _get_module(SPC=8)
    in_maps = make_in_maps(inputs)
    res = run_bass_kernel_spmd(nc, in_maps, core_ids=list(range(N_CORES)))
    return np.concatenate([r["out"] for r in res.results], axis=0)

